# revision 42
# baseline (speedup 1.0000x reference)
"""Multi-head causal attention block on 8 Trainium2 NeuronCores (v2).

Problem: B=4, S=2048, E=1024, H=16, D=64, causal mask, f32.
Sharding: batch (4) x head-group (2 groups of 8 heads) -> 8 cores.
Core c handles batch b=c//2, heads [8g, 8g+8) with g=c%2.
Megatron layout: qkv col-parallel, out_proj row-parallel; the row-parallel
all-reduce (sum of the 2 head-group partial outputs per batch) is done on
host during the gather, as is the output bias.

v2 changes vs baseline (PE was 78% busy, ACT-exp bound attention loop):
  - scores PSUM tiles hold BOTH heads per k-tile ([128, 2*512]) so the
    2-deep ring gives one k-tile of lookahead; one exp instruction covers
    both heads via a [p, 2, w] strided AP.
  - attn@V chases scores by one k-tile so PE rarely waits on the current
    exp. PSUM accumulation groups are PER BANK: each (head, qt)
    accumulator is one start(kt==0)..stop(kt==kt_last) chain of matmuls
    over the live (causal) column range.
  - causal diagonal blocks are narrowed: scores/exp/attn@V only touch
    q >= 128*dlt within the block; the 128-wide true-diagonal sub-block
    gets a [128,128] additive tri-mask folded in via one PE matmul
    (ident stationary). ~17% less PE and ~15% less ACT-exp work.
  - softmax normalization is split: reciprocal + SBUF staging (frees the
    PSUM accumulator) run at the qt boundary; the K=1 reciprocal
    broadcast matmuls + the final multiply are deferred into the filler
    queue. Matmul PSUM destinations must start at partition 0, and DVE
    outputs must start at a 0/32/64/96 partition.
  - a credit-based scheduler (ACT-cost minus PE-cost drip + a per-qt
    exp-latency allowance) interleaves projection matmuls (V tiles, next
    pair's QK projection, output projection) into the ACT-bound
    attention stream as PE filler, with force-drain deadlines one block
    ahead of first use so DVE evacuations have slack.
  - inputs load with ONE wide DMA each (e-tiles merged along the free
    dim; xk split only at column 512 so the prologue's first chains
    unblock after ~2MB); output staged per 128-row block and written
    with one [128,1024] DMA each. GPSIMD cannot access PSUM, so all
    PSUM evacuation stays on DVE.
"""

import numpy as np

B, S, E, H, D = 4, 2048, 1024, 16, 64
HG = H // 2          # heads per group (8)
NP = HG // 2         # head pairs per group (4)
N_CORES = 8
QT_N = 512           # q tile (free dim) in attention
KT_P = 128           # k tile (partitions) in attention
N_QT = S // QT_N     # 4
N_KT = S // KT_P     # 16
F = HG * D           # local features per core (512)

_CACHE = {}

# emission-time engine cost estimates (ns) for the greedy interleaver
_PE_NS = 0.4167
_ACT_NS = 0.833
_ACT_OVH = 170.0


def _build(n_et, repeat=1, phases="abc"):
    phases, _, flags = phases.partition("!")
    import concourse.mybir as mybir
    import concourse.tile as tile
    from concourse import bacc

    dt = mybir.dt
    f32, f32r, bf16 = dt.float32, dt.float32r, dt.bfloat16
    fp8 = dt.float8e4
    DR = mybir.MatmulPerfMode.DoubleRow
    AF = mybir.ActivationFunctionType
    E_pad = n_et * 128

    nc = bacc.Bacc("TRN2", target_bir_lowering=False, debug=False,
                   enable_asserts=False, num_devices=N_CORES)

    XT = nc.dram_tensor("xt", [E_pad, S], bf16, kind="ExternalInput").ap()
    WQKV = nc.dram_tensor("wqkv", [E_pad, 3 * F], bf16, kind="ExternalInput").ap()
    WOUT = nc.dram_tensor("wout", [F, E], bf16, kind="ExternalInput").ap()
    TRI = nc.dram_tensor("tri", [128, 128], bf16, kind="ExternalInput").ap()
    SEL2 = nc.dram_tensor("sel2", [1, 64], f32r, kind="ExternalInput").ap()
    IDENT = nc.dram_tensor("ident", [128, 128], bf16, kind="ExternalInput").ap()
    Y = nc.dram_tensor("y", [S, E], f32, kind="ExternalOutput").ap()

    with tile.TileContext(nc) as tc, \
         nc.allow_low_precision(reason="bf16 matmul operands by design"):
      for _rep in range(repeat):
        with tc.tile_pool(name="persist", bufs=1) as persist, \
             tc.tile_pool(name="mm_ps", bufs=2, space="PSUM") as mm_ps, \
             tc.tile_pool(name="sp_ps", bufs=2, space="PSUM") as sp_ps, \
             tc.tile_pool(name="at_ps", bufs=1, space="PSUM") as at_ps, \
             tc.tile_pool(name="exp_sb", bufs=6) as exp_sb, \
             tc.tile_pool(name="nrm_sb", bufs=2) as nrm_sb, \
             tc.tile_pool(name="y_sb", bufs=4) as y_sbp:
            # ---- persistent SBUF tensors (e-tiles merged along the free
            # dim so each input loads with ONE wide DMA -- per-DMA queue
            # overhead is ~0.6us and the startup path is DMA-latency bound)
            # xk split at column 512 into two tensors: every reader slice
            # is 512-aligned, so no reader spans the boundary, and the
            # first 512 columns (one DMA) unblock the V/QK prologue early
            xk_a = persist.tile([128, n_et, 512], bf16, tag="xka", name="xk_a")
            xk_b = persist.tile([128, n_et, S - 512], bf16, tag="xkb",
                                name="xk_b")
            wqk_t = persist.tile([128, n_et, 2 * F], bf16, tag="wqk",
                                 name="wqk_t")
            wv_t = persist.tile([128, n_et, F], bf16, tag="wv", name="wv_t")
            wout_t = persist.tile([128, NP, E], bf16, tag="wo", name="wout_t")

            def xcol(e, a, b):
                if b <= 512:
                    return xk_a[:, e, a:b]
                return xk_b[:, e, a - 512:b - 512]
            wqk = [wqk_t[:, e, :] for e in range(n_et)]
            wv = [wv_t[:, e, :] for e in range(n_et)]
            wout_sb = [wout_t[:, p, :] for p in range(NP)]
            # Q/K for scores: fp8 staging [128 d, S] (direct DVE evacuation
            # target) + DoubleRow pair layout [64, 2, S] produced by an
            # SBUF->SBUF DMA remap (partition 2p'+t -> (p', t)); head hh sits
            # at pair-partitions [32hh, 32hh+32) with d = 2(p'-32hh)+t.
            qt_s8 = [persist.tile([128, S], fp8, tag=f"qs8{p}", name=f"qs8{p}")
                     for p in range(NP)]
            kt_s8 = [persist.tile([128, S], fp8, tag=f"ks8{p}", name=f"ks8{p}")
                     for p in range(NP)]
            qt_p8 = [persist.tile([64, 2, S], fp8, tag=f"qp8{p}",
                                  name=f"qp8{p}") for p in range(NP)]
            kt_p8 = [persist.tile([64, 2, S], fp8, tag=f"kp8{p}",
                                  name=f"kp8{p}") for p in range(NP)]
            vav = [persist.tile([128, HG * (D + 1)], bf16, tag=f"va{k}",
                                name=f"va{k}") for k in range(N_KT)]
            outt = [persist.tile([128, S], bf16, tag=f"ot{p}", name=f"ot{p}")
                    for p in range(NP)]
            tri_sb = persist.tile([128, 128], bf16, tag="tri")
            sel_sb = persist.tile([1, 64], f32r, tag="sel")
            ident = persist.tile([128, 128], bf16, tag="ident")

            # ---- DMA loads, ordered by first use ----
            XTv = XT.rearrange("(e p) s -> p e s", p=128)
            WQKVv = WQKV.rearrange("(e p) f -> p e f", p=128)
            WOUTv = WOUT.rearrange("(q p) e2 -> p q e2", p=128)
            nc.sync.dma_start(wv_t[:], WQKVv[:, :, 2 * F:3 * F])
            nc.sync.dma_start(xk_a[:], XTv[:, :, 0:512])
            nc.sync.dma_start(wqk_t[:], WQKVv[:, :, 0:2 * F])
            nc.sync.dma_start(tri_sb[:], TRI[:])
            nc.sync.dma_start(sel_sb[:], SEL2[:])
            nc.sync.dma_start(ident[:], IDENT[:])
            nc.sync.dma_start(xk_b[:], XTv[:, :, 512:2048])
            nc.sync.dma_start(wout_t[:], WOUTv[:])

            # ================= filler machinery =================
            # A filler chain is a list of thunks; all but the last emit one
            # PE matmul (returning its moving width); the last emits the
            # PSUM evacuation (DVE) and returns 0.
            state = {"credit": 0.0}
            queue = []   # list of (deadline_block, thunks), kept sorted
            late = []    # chains held back until the final attention block
            ysb_box = {}  # st -> shared [128, E] output staging tile

            def v_chain(st):
                ps_box = {}
                def mk(e):
                    def emit():
                        if e == 0:
                            ps_box["ps"] = mm_ps.tile([128, 512], f32,
                                                      tag="mm", name="mmps")
                        nc.tensor.matmul(
                            ps_box["ps"][:],
                            xcol(e, 128 * st, 128 * (st + 1)),
                            wv[e][:],
                            start=(e == 0), stop=(e == n_et - 1))
                        return 512
                    return emit
                def fin():
                    va3 = vav[st].rearrange("p (h c) -> p h c", c=D + 1)
                    nc.vector.tensor_copy(
                        va3[:, :, 0:D],
                        ps_box["ps"][:].rearrange("p (h c) -> p h c", c=D))
                    nc.any.memset(va3[:, :, D:D + 1], 1.0)
                    return 0
                return [mk(e) for e in range(n_et)] + [fin]

            def qk_chain(p, dest_is_q, sc):
                ft = (p if dest_is_q else NP + p)
                stage = (qt_s8 if dest_is_q else kt_s8)[p]
                pair = (qt_p8 if dest_is_q else kt_p8)[p]
                ps_box = {}
                def mk(e):
                    def emit():
                        if e == 0:
                            ps_box["ps"] = mm_ps.tile([128, 512], f32,
                                                      tag="mm", name="mmps")
                        nc.tensor.matmul(
                            ps_box["ps"][:],
                            wqk[e][:, 128 * ft:128 * (ft + 1)],
                            xcol(e, 512 * sc, 512 * (sc + 1)),
                            start=(e == 0), stop=(e == n_et - 1))
                        return 512
                    return emit
                def fin():
                    nc.vector.tensor_copy(stage[:, 512 * sc:512 * (sc + 1)],
                                          ps_box["ps"][:])
                    # remap [128 d, 512] -> [64, 2, 512] (pair layout) on the
                    # DMA engines; row-major flattening pairs partitions
                    # (2p', 2p'+1) into (p', t)
                    nc.sync.dma_start(pair[:, :, 512 * sc:512 * (sc + 1)],
                                      stage[:, 512 * sc:512 * (sc + 1)])
                    return 0
                return [mk(e) for e in range(n_et)] + [fin]

            def proj_chain(st, et):
                ps_box = {}
                def mk(p):
                    def emit():
                        if p == 0:
                            ps_box["ps"] = mm_ps.tile([128, 512], f32,
                                                      tag="mm", name="mmps")
                            if et == 0:
                                ps_box["ysb"] = y_sbp.tile(
                                    [128, E], f32, tag="ysb", name="ysb")
                                ysb_box[st] = ps_box["ysb"]
                            else:
                                ps_box["ysb"] = ysb_box.pop(st)
                        nc.tensor.matmul(
                            ps_box["ps"][:],
                            outt[p][:, 128 * st:128 * (st + 1)],
                            wout_sb[p][:, 512 * et:512 * (et + 1)],
                            start=(p == 0), stop=(p == NP - 1))
                        return 512
                    return emit
                def fin():
                    ysb = ps_box["ysb"]
                    if st >= 12:
                        # the tail is DVE-backlogged and ACT is idle there
                        nc.scalar.copy(ysb[:, 512 * et:512 * (et + 1)],
                                       ps_box["ps"][:])
                    else:
                        nc.vector.tensor_copy(ysb[:, 512 * et:512 * (et + 1)],
                                              ps_box["ps"][:])
                    if et == 1:
                        nc.sync.dma_start(Y[128 * st:128 * (st + 1), :],
                                          ysb[:])
                    return 0
                return [mk(p) for p in range(NP)] + [fin]

            def drain_one():
                """Emit one thunk from the front chain; True if emitted."""
                while queue and not queue[0][1]:
                    queue.pop(0)
                if not queue:
                    return False
                w = queue[0][1].pop(0)()
                state["credit"] -= w * _PE_NS
                if not queue[0][1]:
                    queue.pop(0)
                return True

            def force_drain(deadline):
                while queue and queue[0][0] <= deadline:
                    _, thunks = queue[0]
                    while thunks:
                        w = thunks.pop(0)()
                        state["credit"] -= w * _PE_NS
                    queue.pop(0)

            def greedy_drain():
                # cap windup so a long ACT-bound stretch can't defer a
                # whole phase of filler into one spot
                if state["credit"] > 4000.0:
                    state["credit"] = 4000.0
                while state["credit"] > 0 and drain_one():
                    pass

            def boundary_fill(ns):
                """Emit ~ns of filler regardless of credit: PE work to
                cover the ACT exp backlog while the sp ring drains at a
                qt boundary."""
                start = state["credit"]
                while start - state["credit"] < ns and drain_one():
                    pass

            def pe_cost(w):
                state["credit"] -= w * _PE_NS

            def act_cost(cols):
                state["credit"] += cols * _ACT_NS + _ACT_OVH

            # ================= attention stream =================
            # chase state: attnV trails scores by one k-tile
            pend = {"av": None, "nrm": None}

            def emit_scores(p, qt, kt, sp, ep):
                """scores for both heads of pair p into sp [128, 2*512];
                exp into ep. Returns nothing; accounts credit."""
                dlt = kt - 4 * qt
                diag = dlt >= 0 and "nomask" not in flags
                sp3 = sp[:].rearrange("p (h w) -> p h w", h=2)
                for hh in range(2):
                    lo, hi = 32 * hh, 32 * hh + 32
                    kst = kt_p8[p][lo:hi, :, 128 * kt:128 * (kt + 1)]
                    qmv = qt_p8[p]
                    half = sp3[:, hh, :]
                    if not diag:
                        nc.tensor.matmul(
                            half,
                            kst,
                            qmv[lo:hi, :, QT_N * qt:QT_N * (qt + 1)],
                            start=True, stop=True, perf_mode=DR)
                        pe_cost(256)
                    else:
                        lv = 128 * dlt   # live q cols [lv:512)
                        if lv + 128 < 512:
                            nc.tensor.matmul(
                                half[:, lv + 128:512],
                                kst,
                                qmv[lo:hi, :,
                                    QT_N * qt + lv + 128:QT_N * (qt + 1)],
                                start=True, stop=True, perf_mode=DR)
                            pe_cost((512 - lv - 128) // 2)
                        nc.tensor.matmul(
                            half[:, lv:lv + 128],
                            kst,
                            qmv[lo:hi, :,
                                QT_N * qt + lv:QT_N * qt + lv + 128],
                            start=True, stop=False, perf_mode=DR)
                        nc.tensor.matmul(
                            half[:, lv:lv + 128], ident[:], tri_sb[:],
                            start=False, stop=True)
                        pe_cost(192)
                lv = max(0, 128 * dlt) if "nomask" not in flags else 0
                ep3 = ep[:].rearrange("p (h w) -> p h w", h=2)
                nc.scalar.activation(
                    ep3[:, :, lv:512],
                    sp3[:, :, lv:512],
                    AF.Copy if "noexp" in flags else AF.Exp,
                    scale=float(1.0 / np.sqrt(D)))
                act_cost(2 * (512 - lv))

            def emit_attnv(p, qt, kt, ep, aps):
                # ONE matmul per (head, k-tile) over the live column range:
                # PSUM accumulation groups are per-bank, so the group must
                # be a single start(kt==0) ... stop(kt==kt_last) chain
                dlt = kt - 4 * qt
                kt_last = 4 * qt + 3
                lv = max(0, 128 * dlt)
                ep3 = ep[:].rearrange("p (h w) -> p h w", h=2)
                for hh, ap in ((0, aps[0]), (1, aps[1])):
                    h = 2 * p + hh
                    vsl = vav[kt][:, (D + 1) * h:(D + 1) * (h + 1)]
                    nc.tensor.matmul(
                        ap[0:D + 1, lv:512], vsl, ep3[:, hh, lv:512],
                        start=(kt == 0), stop=(kt == kt_last))
                    pe_cost(512 - lv)

            def emit_normalize(p, qt, aps, g):
                """One wide DVE evacuation per head (values + denominator
                row together, frees the PSUM bank) + a fast approximate
                reciprocal of the denominator row. The row-broadcast (K=1
                matmul against a ones column) and the final multiply are
                deferred into the filler queue as in v2 -- the PE broadcast
                is the only HW-proven way to expand a [1,N] row across
                partitions."""
                stgs, recs = [], []
                for hh, ap in ((0, aps[0]), (1, aps[1])):
                    stg = nrm_sb.tile([65, QT_N], f32, tag=f"stgf{hh}",
                                      name=f"stgf{hh}")
                    nc.vector.tensor_copy(stg[:], ap[0:D + 1, :])
                    recr = nrm_sb.tile([1, QT_N], f32r, tag=f"recr{hh}",
                                       name=f"recr{hh}")
                    nc.vector.reciprocal(recr[:], stg[64:65, :])
                    stgs.append(stg)
                    recs.append(recr)
                ps_box = {}
                def t_bcast(hh):
                    def emit():
                        ps_box[hh] = mm_ps.tile([64, QT_N], f32,
                                                tag="mm", name="mmps")
                        nc.tensor.matmul(ps_box[hh][:], sel_sb[:],
                                         recs[hh][:], start=True, stop=True)
                        return 512
                    return emit
                def t_mul(hh):
                    def emit():
                        nc.vector.tensor_tensor(
                            outt[p][64 * hh:64 * hh + 64,
                                    QT_N * qt:QT_N * (qt + 1)],
                            stgs[hh][0:64, :], ps_box[hh][:],
                            mybir.AluOpType.mult)
                        return 0
                    return emit
                queue.append((g + 1, [t_bcast(0), t_mul(0),
                                      t_bcast(1), t_mul(1)]))
                queue.sort(key=lambda it: it[0])

            def flush_pending():
                if pend["av"] is not None:
                    emit_attnv(*pend["av"])
                    pend["av"] = None
                if pend["nrm"] is not None:
                    p_, qt_, aps_ = pend["nrm"]
                    emit_normalize(p_, qt_, aps_, 4 * p_ + qt_)
                    pend["nrm"] = None
                    if p_ == NP - 1 and phases == "abc":
                        for st in range(4 * qt_, 4 * (qt_ + 1)):
                            for et in range(E // 512):
                                queue.append((NP * N_QT, proj_chain(st, et)))

            # ---- enqueue fillers with deadlines (scalar attention-block
            # index g = 4*p + qt). QK chains deadline one block EARLY so
            # their DVE evacuation has a whole qt of slack before the
            # first scores matmul that reads them.
            for st in range(N_KT):
                queue.append((st // 4, v_chain(st)))
            for p in range(NP):
                for sc in range(N_QT):
                    queue.append((max(4 * p + sc - 1, 0), qk_chain(p, False, sc)))
                    queue.append((max(4 * p + sc - 1, 0), qk_chain(p, True, sc)))
            queue.sort(key=lambda it: it[0])

            if phases == "a":
                force_drain(NP * N_QT)
                continue

            for p in range(NP):
                for qt in range(N_QT):
                    force_drain(4 * p + qt)
                    # filler first: PE covers the ACT exp backlog while the
                    # trailing attn@V (flushed next) is still exp-blocked
                    boundary_fill(1500.0)
                    flush_pending()
                    # local credit: a fresh exp-latency ramp allowance; the
                    # per-kt surplus (ACT cost - PE cost) then drips filler
                    # at exactly the rate PE would otherwise idle
                    state["credit"] = 1200.0
                    aps = (at_ps.tile([128, QT_N], f32, tag="apA", name="apA"),
                           at_ps.tile([128, QT_N], f32, tag="apB", name="apB"))
                    for kt in range(4 * qt + 4):
                        sp = sp_ps.tile([128, 2 * QT_N], f32, tag="sp",
                                        name="sp")
                        ep = exp_sb.tile([128, 2 * QT_N], bf16, tag="ep",
                                         name="ep")
                        emit_scores(p, qt, kt, sp, ep)
                        # fillers BEFORE the (possibly exp-stalled) attn@V:
                        # the PE wait-queue is 4 deep, and instructions
                        # behind a full wait-queue cannot decode at all
                        greedy_drain()
                        flush_pending()
                        pend["av"] = (p, qt, kt, ep, aps)
                    pend["nrm"] = (p, qt, aps)
            flush_pending()
            force_drain(NP * N_QT)

            if phases == "ab":
                for p in range(NP):
                    for half in range(2):
                        nc.sync.dma_start(
                            Y[(2 * p + half) * 128:(2 * p + half + 1) * 128, :]
                            .bitcast(bf16),
                            outt[p][:, :])

    nc.compile()
    return nc


def _get_nc(n_et, repeat=1, phases="abc"):
    key = (n_et, repeat, phases)
    if key not in _CACHE:
        _CACHE[key] = _build(n_et, repeat, phases)
    return _CACHE[key]


def _shard(x, mask, Wqkv, bqkv, Wout, bout):
    """Host-side sharding: per-core input dicts."""
    import ml_dtypes

    bf16 = ml_dtypes.bfloat16
    x = np.asarray(x, dtype=np.float32)
    Wqkv = np.asarray(Wqkv, dtype=np.float32)
    bqkv = np.asarray(bqkv, dtype=np.float32)
    Wout = np.asarray(Wout, dtype=np.float32)

    has_bias = bool(np.any(bqkv))
    n_et = 9 if has_bias else 8
    E_pad = n_et * 128

    # additive causal tri mask for the 128x128 true-diagonal block:
    # tri[i, j] masks scoresT[k=i, q=j]: 0 where j >= i, -240 otherwise.
    ii, jj = np.meshgrid(np.arange(128), np.arange(128), indexing="ij")
    tri = np.where(jj >= ii, 0.0, -240.0).astype(bf16)
    tri = np.ascontiguousarray(tri)

    sel2 = np.ones((1, 64), np.float32)

    in_maps = []
    for c in range(N_CORES):
        b, g = divmod(c, 2)
        heads = range(HG * g, HG * (g + 1))
        cols = []
        for blk in range(3):  # q, k, v blocks of Wqkv
            for h in heads:
                cols.append(Wqkv[:, blk * E + D * h: blk * E + D * h + D])
        wqkv_c = np.concatenate(cols, axis=1)  # [E, 3F]
        if has_bias:
            bias_cols = []
            for blk in range(3):
                for h in heads:
                    bias_cols.append(bqkv[blk * E + D * h: blk * E + D * h + D])
            brow = np.concatenate(bias_cols)[None, :]  # [1, 3F]
            wqkv_c = np.concatenate(
                [wqkv_c, brow, np.zeros((E_pad - E - 1, 3 * F), np.float32)], axis=0)
        xt_c = np.ascontiguousarray(x[b].T)  # [E, S]
        if has_bias:
            aug = np.zeros((E_pad - E, S), np.float32)
            aug[0, :] = 1.0
            xt_c = np.concatenate([xt_c, aug], axis=0)
        wout_c = np.ascontiguousarray(Wout[F * g:F * (g + 1), :])  # [F, E]
        in_maps.append({
            "xt": np.ascontiguousarray(xt_c.astype(bf16)),
            "wqkv": np.ascontiguousarray(wqkv_c.astype(bf16)),
            "wout": np.ascontiguousarray(wout_c.astype(bf16)),
            "tri": tri,
            "sel2": sel2,
            "ident": np.eye(128, dtype=bf16),
        })
    return in_maps, n_et


def run_sharded(inputs, trace=False):
    """Run the SPMD kernel; returns (y_full [B,S,E] f32, BassKernelResults)."""
    from concourse.bass_utils import run_bass_kernel_spmd

    in_maps, n_et = _shard(**inputs)
    nc = _get_nc(n_et)
    res = run_bass_kernel_spmd(nc, in_maps, core_ids=list(range(N_CORES)),
                               trace=trace)
    bout = np.asarray(inputs["bout"], dtype=np.float32)
    y = np.empty((B, S, E), np.float32)
    for b in range(B):
        y[b] = (res.results[2 * b]["y"] + res.results[2 * b + 1]["y"] + bout)
    return y, res


def kernel(**inputs) -> np.ndarray:
    y, _ = run_sharded(inputs, trace=False)
    return y



# revision 47
# speedup vs baseline: 1.0301x; 1.0301x over previous
"""Multi-head causal attention block on 8 Trainium2 NeuronCores (v2).

Problem: B=4, S=2048, E=1024, H=16, D=64, causal mask, f32.
Sharding: batch (4) x head-group (2 groups of 8 heads) -> 8 cores.
Core c handles batch b=c//2, heads [8g, 8g+8) with g=c%2.
Megatron layout: qkv col-parallel, out_proj row-parallel; the row-parallel
all-reduce (sum of the 2 head-group partial outputs per batch) is done on
host during the gather, as is the output bias.

v2 changes vs baseline (PE was 78% busy, ACT-exp bound attention loop):
  - scores PSUM tiles hold BOTH heads per k-tile ([128, 2*512]) so the
    2-deep ring gives one k-tile of lookahead; one exp instruction covers
    both heads via a [p, 2, w] strided AP.
  - attn@V chases scores by one k-tile so PE rarely waits on the current
    exp. PSUM accumulation groups are PER BANK: each (head, qt)
    accumulator is one start(kt==0)..stop(kt==kt_last) chain of matmuls
    over the live (causal) column range.
  - causal diagonal blocks are narrowed: scores/exp/attn@V only touch
    q >= 128*dlt within the block; the 128-wide true-diagonal sub-block
    gets a [128,128] additive tri-mask folded in via one PE matmul
    (ident stationary). ~17% less PE and ~15% less ACT-exp work.
  - softmax normalization is split: reciprocal + SBUF staging (frees the
    PSUM accumulator) run at the qt boundary; the K=1 reciprocal
    broadcast matmuls + the final multiply are deferred into the filler
    queue. Matmul PSUM destinations must start at partition 0, and DVE
    outputs must start at a 0/32/64/96 partition.
  - a credit-based scheduler (ACT-cost minus PE-cost drip + a per-qt
    exp-latency allowance) interleaves projection matmuls (V tiles, next
    pair's QK projection, output projection) into the ACT-bound
    attention stream as PE filler, with force-drain deadlines one block
    ahead of first use so DVE evacuations have slack.
  - inputs load with ONE wide DMA each (e-tiles merged along the free
    dim; xk split only at column 512 so the prologue's first chains
    unblock after ~2MB); output staged per 128-row block and written
    with one [128,1024] DMA each. GPSIMD cannot access PSUM, so all
    PSUM evacuation stays on DVE.
"""

import numpy as np

B, S, E, H, D = 4, 2048, 1024, 16, 64
HG = H // 2          # heads per group (8)
NP = HG // 2         # head pairs per group (4)
N_CORES = 8
QT_N = 512           # q tile (free dim) in attention
KT_P = 128           # k tile (partitions) in attention
N_QT = S // QT_N     # 4
N_KT = S // KT_P     # 16
F = HG * D           # local features per core (512)

_CACHE = {}

# emission-time engine cost estimates (ns) for the greedy interleaver
_PE_NS = 0.4167
_ACT_NS = 0.833
_ACT_OVH = 170.0


def _build(n_et, repeat=1, phases="abc"):
    phases, _, flags = phases.partition("!")
    import concourse.mybir as mybir
    import concourse.tile as tile
    from concourse import bacc

    dt = mybir.dt
    f32, f32r, bf16 = dt.float32, dt.float32r, dt.bfloat16
    f16 = dt.float16
    fp8 = dt.float8e4
    DR = mybir.MatmulPerfMode.DoubleRow
    AF = mybir.ActivationFunctionType
    E_pad = n_et * 128

    nc = bacc.Bacc("TRN2", target_bir_lowering=False, debug=False,
                   enable_asserts=False, num_devices=N_CORES)

    XT = nc.dram_tensor("xt", [E_pad, S], bf16, kind="ExternalInput").ap()
    WQKV = nc.dram_tensor("wqkv", [E_pad, 3 * F], bf16, kind="ExternalInput").ap()
    WOUT = nc.dram_tensor("wout", [F, E], bf16, kind="ExternalInput").ap()
    TRI = nc.dram_tensor("tri", [128, 128], bf16, kind="ExternalInput").ap()
    SEL2 = nc.dram_tensor("sel2", [1, 64], f32r, kind="ExternalInput").ap()
    IDENT = nc.dram_tensor("ident", [128, 128], bf16, kind="ExternalInput").ap()
    Y = nc.dram_tensor("y", [S, E], f16, kind="ExternalOutput").ap()

    with tile.TileContext(nc) as tc, \
         nc.allow_low_precision(reason="bf16 matmul operands by design"):
      for _rep in range(repeat):
        with tc.tile_pool(name="persist", bufs=1) as persist, \
             tc.tile_pool(name="mm_ps", bufs=2, space="PSUM") as mm_ps, \
             tc.tile_pool(name="sp_ps", bufs=2, space="PSUM") as sp_ps, \
             tc.tile_pool(name="at_ps", bufs=1, space="PSUM") as at_ps, \
             tc.tile_pool(name="exp_sb", bufs=6) as exp_sb, \
             tc.tile_pool(name="nrm_sb", bufs=2) as nrm_sb, \
             tc.tile_pool(name="y_sb", bufs=4) as y_sbp:
            # ---- persistent SBUF tensors (e-tiles merged along the free
            # dim so each input loads with ONE wide DMA -- per-DMA queue
            # overhead is ~0.6us and the startup path is DMA-latency bound)
            # xk split at column 512 into two tensors: every reader slice
            # is 512-aligned, so no reader spans the boundary, and the
            # first 512 columns (one DMA) unblock the V/QK prologue early
            xk_a = persist.tile([128, n_et, 512], bf16, tag="xka", name="xk_a")
            xk_b = persist.tile([128, n_et, S - 512], bf16, tag="xkb",
                                name="xk_b")
            wqk_t = persist.tile([128, n_et, 2 * F], bf16, tag="wqk",
                                 name="wqk_t")
            wv_t = persist.tile([128, n_et, F], bf16, tag="wv", name="wv_t")
            wout_t = persist.tile([128, NP, E], bf16, tag="wo", name="wout_t")

            def xcol(e, a, b):
                if b <= 512:
                    return xk_a[:, e, a:b]
                return xk_b[:, e, a - 512:b - 512]
            wqk = [wqk_t[:, e, :] for e in range(n_et)]
            wv = [wv_t[:, e, :] for e in range(n_et)]
            wout_sb = [wout_t[:, p, :] for p in range(NP)]
            # Q/K for scores: fp8 staging [128 d, S] (direct DVE evacuation
            # target) + DoubleRow pair layout [64, 2, S] produced by an
            # SBUF->SBUF DMA remap (partition 2p'+t -> (p', t)); head hh sits
            # at pair-partitions [32hh, 32hh+32) with d = 2(p'-32hh)+t.
            qt_s8 = [persist.tile([128, S], fp8, tag=f"qs8{p}", name=f"qs8{p}")
                     for p in range(NP)]
            kt_s8 = [persist.tile([128, S], fp8, tag=f"ks8{p}", name=f"ks8{p}")
                     for p in range(NP)]
            qt_p8 = [persist.tile([64, 2, S], fp8, tag=f"qp8{p}",
                                  name=f"qp8{p}") for p in range(NP)]
            kt_p8 = [persist.tile([64, 2, S], fp8, tag=f"kp8{p}",
                                  name=f"kp8{p}") for p in range(NP)]
            vav = [persist.tile([128, HG * (D + 1)], bf16, tag=f"va{k}",
                                name=f"va{k}") for k in range(N_KT)]
            outt = [persist.tile([128, S], bf16, tag=f"ot{p}", name=f"ot{p}")
                    for p in range(NP)]
            tri_sb = persist.tile([128, 128], bf16, tag="tri")
            sel_sb = persist.tile([1, 64], f32r, tag="sel")
            ident = persist.tile([128, 128], bf16, tag="ident")

            # ---- DMA loads, ordered by first use ----
            XTv = XT.rearrange("(e p) s -> p e s", p=128)
            WQKVv = WQKV.rearrange("(e p) f -> p e f", p=128)
            WOUTv = WOUT.rearrange("(q p) e2 -> p q e2", p=128)
            nc.sync.dma_start(wv_t[:], WQKVv[:, :, 2 * F:3 * F])
            nc.sync.dma_start(xk_a[:], XTv[:, :, 0:512])
            nc.sync.dma_start(wqk_t[:], WQKVv[:, :, 0:2 * F])
            nc.sync.dma_start(tri_sb[:], TRI[:])
            nc.sync.dma_start(sel_sb[:], SEL2[:])
            nc.sync.dma_start(ident[:], IDENT[:])
            nc.sync.dma_start(xk_b[:], XTv[:, :, 512:2048])
            nc.sync.dma_start(wout_t[:], WOUTv[:])

            # ================= filler machinery =================
            # A filler chain is a list of thunks; all but the last emit one
            # PE matmul (returning its moving width); the last emits the
            # PSUM evacuation (DVE) and returns 0.
            state = {"credit": 0.0}
            queue = []   # list of (deadline_block, thunks), kept sorted
            late = []    # chains held back until the final attention block
            ysb_box = {}  # st -> shared [128, E] output staging tile

            def v_chain(st):
                ps_box = {}
                def mk(e):
                    def emit():
                        if e == 0:
                            ps_box["ps"] = mm_ps.tile([128, 512], f32,
                                                      tag="mm", name="mmps")
                        nc.tensor.matmul(
                            ps_box["ps"][:],
                            xcol(e, 128 * st, 128 * (st + 1)),
                            wv[e][:],
                            start=(e == 0), stop=(e == n_et - 1))
                        return 512
                    return emit
                def fin():
                    va3 = vav[st].rearrange("p (h c) -> p h c", c=D + 1)
                    nc.vector.tensor_copy(
                        va3[:, :, 0:D],
                        ps_box["ps"][:].rearrange("p (h c) -> p h c", c=D))
                    nc.any.memset(va3[:, :, D:D + 1], 1.0)
                    return 0
                return [mk(e) for e in range(n_et)] + [fin]

            def qk_chain(p, dest_is_q, sc):
                ft = (p if dest_is_q else NP + p)
                stage = (qt_s8 if dest_is_q else kt_s8)[p]
                pair = (qt_p8 if dest_is_q else kt_p8)[p]
                ps_box = {}
                def mk(e):
                    def emit():
                        if e == 0:
                            ps_box["ps"] = mm_ps.tile([128, 512], f32,
                                                      tag="mm", name="mmps")
                        nc.tensor.matmul(
                            ps_box["ps"][:],
                            wqk[e][:, 128 * ft:128 * (ft + 1)],
                            xcol(e, 512 * sc, 512 * (sc + 1)),
                            start=(e == 0), stop=(e == n_et - 1))
                        return 512
                    return emit
                def fin():
                    nc.vector.tensor_copy(stage[:, 512 * sc:512 * (sc + 1)],
                                          ps_box["ps"][:])
                    # remap [128 d, 512] -> [64, 2, 512] (pair layout) on the
                    # DMA engines; row-major flattening pairs partitions
                    # (2p', 2p'+1) into (p', t)
                    nc.sync.dma_start(pair[:, :, 512 * sc:512 * (sc + 1)],
                                      stage[:, 512 * sc:512 * (sc + 1)])
                    return 0
                return [mk(e) for e in range(n_et)] + [fin]

            def proj_chain(st, et):
                ps_box = {}
                def mk(p):
                    def emit():
                        if p == 0:
                            ps_box["ps"] = mm_ps.tile([128, 512], f32,
                                                      tag="mm", name="mmps")
                            if et == 0:
                                ps_box["ysb"] = y_sbp.tile(
                                    [128, E], f16, tag="ysb", name="ysb")
                                ysb_box[st] = ps_box["ysb"]
                            else:
                                ps_box["ysb"] = ysb_box.pop(st)
                        nc.tensor.matmul(
                            ps_box["ps"][:],
                            outt[p][:, 128 * st:128 * (st + 1)],
                            wout_sb[p][:, 512 * et:512 * (et + 1)],
                            start=(p == 0), stop=(p == NP - 1))
                        return 512
                    return emit
                def fin():
                    ysb = ps_box["ysb"]
                    if st >= 12:
                        # the tail is DVE-backlogged and ACT is idle there
                        nc.scalar.copy(ysb[:, 512 * et:512 * (et + 1)],
                                       ps_box["ps"][:])
                    else:
                        nc.vector.tensor_copy(ysb[:, 512 * et:512 * (et + 1)],
                                              ps_box["ps"][:])
                    if et == 1:
                        nc.sync.dma_start(Y[128 * st:128 * (st + 1), :],
                                          ysb[:])
                    return 0
                return [mk(p) for p in range(NP)] + [fin]

            def drain_one():
                """Emit one thunk from the front chain; True if emitted."""
                while queue and not queue[0][1]:
                    queue.pop(0)
                if not queue:
                    return False
                w = queue[0][1].pop(0)()
                state["credit"] -= w * _PE_NS
                if not queue[0][1]:
                    queue.pop(0)
                return True

            def force_drain(deadline):
                while queue and queue[0][0] <= deadline:
                    _, thunks = queue[0]
                    while thunks:
                        w = thunks.pop(0)()
                        state["credit"] -= w * _PE_NS
                    queue.pop(0)

            def greedy_drain():
                # cap windup so a long ACT-bound stretch can't defer a
                # whole phase of filler into one spot
                if state["credit"] > 4000.0:
                    state["credit"] = 4000.0
                while state["credit"] > 0 and drain_one():
                    pass

            def boundary_fill(ns):
                """Emit ~ns of filler regardless of credit: PE work to
                cover the ACT exp backlog while the sp ring drains at a
                qt boundary."""
                start = state["credit"]
                while start - state["credit"] < ns and drain_one():
                    pass

            def pe_cost(w):
                state["credit"] -= w * _PE_NS

            def act_cost(cols):
                state["credit"] += cols * _ACT_NS + _ACT_OVH

            # ================= attention stream =================
            # chase state: attnV trails scores by one k-tile
            pend = {"av": None, "nrm": None}

            def emit_scores(p, qt, kt, sp, ep):
                """scores for both heads of pair p into sp [128, 2*512];
                exp into ep. Returns nothing; accounts credit."""
                dlt = kt - 4 * qt
                diag = dlt >= 0 and "nomask" not in flags
                sp3 = sp[:].rearrange("p (h w) -> p h w", h=2)
                for hh in range(2):
                    lo, hi = 32 * hh, 32 * hh + 32
                    kst = kt_p8[p][lo:hi, :, 128 * kt:128 * (kt + 1)]
                    qmv = qt_p8[p]
                    half = sp3[:, hh, :]
                    if not diag:
                        nc.tensor.matmul(
                            half,
                            kst,
                            qmv[lo:hi, :, QT_N * qt:QT_N * (qt + 1)],
                            start=True, stop=True, perf_mode=DR)
                        pe_cost(256)
                    else:
                        lv = 128 * dlt   # live q cols [lv:512)
                        if lv + 128 < 512:
                            nc.tensor.matmul(
                                half[:, lv + 128:512],
                                kst,
                                qmv[lo:hi, :,
                                    QT_N * qt + lv + 128:QT_N * (qt + 1)],
                                start=True, stop=True, perf_mode=DR)
                            pe_cost((512 - lv - 128) // 2)
                        nc.tensor.matmul(
                            half[:, lv:lv + 128],
                            kst,
                            qmv[lo:hi, :,
                                QT_N * qt + lv:QT_N * qt + lv + 128],
                            start=True, stop=False, perf_mode=DR)
                        nc.tensor.matmul(
                            half[:, lv:lv + 128], ident[:], tri_sb[:],
                            start=False, stop=True)
                        pe_cost(192)
                lv = max(0, 128 * dlt) if "nomask" not in flags else 0
                ep3 = ep[:].rearrange("p (h w) -> p h w", h=2)
                nc.scalar.activation(
                    ep3[:, :, lv:512],
                    sp3[:, :, lv:512],
                    AF.Copy if "noexp" in flags else AF.Exp,
                    scale=float(1.0 / np.sqrt(D)))
                act_cost(2 * (512 - lv))

            def emit_attnv(p, qt, kt, ep, aps):
                # ONE matmul per (head, k-tile) over the live column range:
                # PSUM accumulation groups are per-bank, so the group must
                # be a single start(kt==0) ... stop(kt==kt_last) chain
                dlt = kt - 4 * qt
                kt_last = 4 * qt + 3
                lv = max(0, 128 * dlt)
                ep3 = ep[:].rearrange("p (h w) -> p h w", h=2)
                for hh, ap in ((0, aps[0]), (1, aps[1])):
                    h = 2 * p + hh
                    vsl = vav[kt][:, (D + 1) * h:(D + 1) * (h + 1)]
                    nc.tensor.matmul(
                        ap[0:D + 1, lv:512], vsl, ep3[:, hh, lv:512],
                        start=(kt == 0), stop=(kt == kt_last))
                    pe_cost(512 - lv)

            def emit_normalize(p, qt, aps, g):
                """One wide DVE evacuation per head (values + denominator
                row together, frees the PSUM bank) + a fast approximate
                reciprocal of the denominator row. The row-broadcast (K=1
                matmul against a ones column) and the final multiply are
                deferred into the filler queue as in v2 -- the PE broadcast
                is the only HW-proven way to expand a [1,N] row across
                partitions."""
                stgs, recs = [], []
                for hh, ap in ((0, aps[0]), (1, aps[1])):
                    stg = nrm_sb.tile([65, QT_N], f32, tag=f"stgf{hh}",
                                      name=f"stgf{hh}")
                    nc.vector.tensor_copy(stg[:], ap[0:D + 1, :])
                    recr = nrm_sb.tile([1, QT_N], f32r, tag=f"recr{hh}",
                                       name=f"recr{hh}")
                    nc.vector.reciprocal(recr[:], stg[64:65, :])
                    stgs.append(stg)
                    recs.append(recr)
                ps_box = {}
                def t_bcast(hh):
                    def emit():
                        ps_box[hh] = mm_ps.tile([64, QT_N], f32,
                                                tag="mm", name="mmps")
                        nc.tensor.matmul(ps_box[hh][:], sel_sb[:],
                                         recs[hh][:], start=True, stop=True)
                        return 512
                    return emit
                def t_mul(hh):
                    def emit():
                        nc.vector.tensor_tensor(
                            outt[p][64 * hh:64 * hh + 64,
                                    QT_N * qt:QT_N * (qt + 1)],
                            stgs[hh][0:64, :], ps_box[hh][:],
                            mybir.AluOpType.mult)
                        return 0
                    return emit
                queue.append((g + 1, [t_bcast(0), t_mul(0),
                                      t_bcast(1), t_mul(1)]))
                queue.sort(key=lambda it: it[0])

            def flush_pending():
                if pend["av"] is not None:
                    emit_attnv(*pend["av"])
                    pend["av"] = None
                if pend["nrm"] is not None:
                    p_, qt_, aps_ = pend["nrm"]
                    emit_normalize(p_, qt_, aps_, 4 * p_ + qt_)
                    pend["nrm"] = None
                    if p_ == NP - 1 and phases == "abc":
                        for st in range(4 * qt_, 4 * (qt_ + 1)):
                            for et in range(E // 512):
                                queue.append((NP * N_QT, proj_chain(st, et)))

            # ---- enqueue fillers with deadlines (scalar attention-block
            # index g = 4*p + qt). QK chains deadline one block EARLY so
            # their DVE evacuation has a whole qt of slack before the
            # first scores matmul that reads them.
            for st in range(N_KT):
                queue.append((st // 4, v_chain(st)))
            for p in range(NP):
                for sc in range(N_QT):
                    queue.append((max(4 * p + sc - 1, 0), qk_chain(p, False, sc)))
                    queue.append((max(4 * p + sc - 1, 0), qk_chain(p, True, sc)))
            queue.sort(key=lambda it: it[0])

            if phases == "a":
                force_drain(NP * N_QT)
                continue

            for p in range(NP):
                for qt in range(N_QT):
                    force_drain(4 * p + qt)
                    # filler first: PE covers the ACT exp backlog while the
                    # trailing attn@V (flushed next) is still exp-blocked
                    boundary_fill(1500.0)
                    flush_pending()
                    # local credit: a fresh exp-latency ramp allowance; the
                    # per-kt surplus (ACT cost - PE cost) then drips filler
                    # at exactly the rate PE would otherwise idle
                    state["credit"] = 1200.0
                    aps = (at_ps.tile([128, QT_N], f32, tag="apA", name="apA"),
                           at_ps.tile([128, QT_N], f32, tag="apB", name="apB"))
                    for kt in range(4 * qt + 4):
                        sp = sp_ps.tile([128, 2 * QT_N], f32, tag="sp",
                                        name="sp")
                        ep = exp_sb.tile([128, 2 * QT_N], bf16, tag="ep",
                                         name="ep")
                        emit_scores(p, qt, kt, sp, ep)
                        # fillers BEFORE the (possibly exp-stalled) attn@V:
                        # the PE wait-queue is 4 deep, and instructions
                        # behind a full wait-queue cannot decode at all
                        greedy_drain()
                        flush_pending()
                        pend["av"] = (p, qt, kt, ep, aps)
                    pend["nrm"] = (p, qt, aps)
            flush_pending()
            force_drain(NP * N_QT)

            if phases == "ab":
                for p in range(NP):
                    for half in range(2):
                        nc.sync.dma_start(
                            Y[(2 * p + half) * 128:(2 * p + half + 1) * 128, :]
                            .bitcast(bf16),
                            outt[p][:, 0:1024])

    nc.compile()
    return nc


def _get_nc(n_et, repeat=1, phases="abc"):
    key = (n_et, repeat, phases)
    if key not in _CACHE:
        _CACHE[key] = _build(n_et, repeat, phases)
    return _CACHE[key]


def _shard(x, mask, Wqkv, bqkv, Wout, bout):
    """Host-side sharding: per-core input dicts."""
    import ml_dtypes

    bf16 = ml_dtypes.bfloat16
    x = np.asarray(x, dtype=np.float32)
    Wqkv = np.asarray(Wqkv, dtype=np.float32)
    bqkv = np.asarray(bqkv, dtype=np.float32)
    Wout = np.asarray(Wout, dtype=np.float32)

    has_bias = bool(np.any(bqkv))
    n_et = 9 if has_bias else 8
    E_pad = n_et * 128

    # additive causal tri mask for the 128x128 true-diagonal block:
    # tri[i, j] masks scoresT[k=i, q=j]: 0 where j >= i, -240 otherwise.
    ii, jj = np.meshgrid(np.arange(128), np.arange(128), indexing="ij")
    tri = np.where(jj >= ii, 0.0, -240.0).astype(bf16)
    tri = np.ascontiguousarray(tri)

    sel2 = np.ones((1, 64), np.float32)

    in_maps = []
    for c in range(N_CORES):
        b, g = divmod(c, 2)
        heads = range(HG * g, HG * (g + 1))
        cols = []
        for blk in range(3):  # q, k, v blocks of Wqkv
            for h in heads:
                cols.append(Wqkv[:, blk * E + D * h: blk * E + D * h + D])
        wqkv_c = np.concatenate(cols, axis=1)  # [E, 3F]
        if has_bias:
            bias_cols = []
            for blk in range(3):
                for h in heads:
                    bias_cols.append(bqkv[blk * E + D * h: blk * E + D * h + D])
            brow = np.concatenate(bias_cols)[None, :]  # [1, 3F]
            wqkv_c = np.concatenate(
                [wqkv_c, brow, np.zeros((E_pad - E - 1, 3 * F), np.float32)], axis=0)
        xt_c = np.ascontiguousarray(x[b].T)  # [E, S]
        if has_bias:
            aug = np.zeros((E_pad - E, S), np.float32)
            aug[0, :] = 1.0
            xt_c = np.concatenate([xt_c, aug], axis=0)
        wout_c = np.ascontiguousarray(Wout[F * g:F * (g + 1), :])  # [F, E]
        in_maps.append({
            "xt": np.ascontiguousarray(xt_c.astype(bf16)),
            "wqkv": np.ascontiguousarray(wqkv_c.astype(bf16)),
            "wout": np.ascontiguousarray(wout_c.astype(bf16)),
            "tri": tri,
            "sel2": sel2,
            "ident": np.eye(128, dtype=bf16),
        })
    return in_maps, n_et


def run_sharded(inputs, trace=False):
    """Run the SPMD kernel; returns (y_full [B,S,E] f32, BassKernelResults)."""
    from concourse.bass_utils import run_bass_kernel_spmd

    in_maps, n_et = _shard(**inputs)
    nc = _get_nc(n_et)
    res = run_bass_kernel_spmd(nc, in_maps, core_ids=list(range(N_CORES)),
                               trace=trace)
    bout = np.asarray(inputs["bout"], dtype=np.float32)
    y = np.empty((B, S, E), np.float32)
    for b in range(B):
        y[b] = (res.results[2 * b]["y"].astype(np.float32)
                + res.results[2 * b + 1]["y"].astype(np.float32) + bout)
    return y, res


def kernel(**inputs) -> np.ndarray:
    y, _ = run_sharded(inputs, trace=False)
    return y

